# revision 44
# baseline (speedup 1.0000x reference)
"""Trainium2 Bass kernel for nn_CodedNet (roll -> binary mask -> unroll -> channel sum).

Math simplification: the forward roll by -ch, the 64x64 binary mask multiply,
and the backward roll by +ch collapse to

    out[b,i,w] = sum_ch x[b,i,w,ch] * mask32[(i-ch)%32, w%32]

where mask32 = sign(w_in).reshape(32,32)  (the 64x64 mask is a 2x2 tile of it).

Strategy (shipped VARIANT="v7ip3"): pure data parallel over batch (512 ->
64 per core on 8 cores). Per core, 16 fused tiles of [128 partitions =
2 batches x 64 rows, 3968 free = 2 batch-pairs x 64 w x 31 ch]:

  sync-ring HWDGE: 2x 1MB fp32 DMA in (6-deep staging pool; the
                   bottleneck: x stream)
  ACT:             copy-cast fp32 -> bf16, one op per 1MB half so the
                   cast starts as soon as the first half lands (53us,
                   hidden under the DMA stream)
  DVE:             bf16 tensor_mul by sign mask, in-place into the bf16
                   tile (2x_1P mode, 33us; in-place drops the prod pool,
                   ~1.6% faster than a separate output tile)
  DVE:             bf16 segmented reduce_sum    (2x_1P mode, 33us)
  ACT:             cast red bf16 -> fp32, out DMA on the scalar ring

The fp32 path (v3) was DVE-bound at 131.5us (fp32 tensor ops run at 1x =
123 G elem/s; two passes over 8.1M elems/core = 132us). Moving the cast to
the idle ACT engine and doing mul+reduce in bf16 at 2x cuts DVE to ~66us,
under the ~94us DMA floor. Measured (repetition-amplified, pipelined-burst
timing): 98.8us/body = 34.5 MB/core / 98.8us = 349 GB/s, 97.5% of the
358 GB/s per-NeuronCore HBM limit. bf16 rel_err 3.1e-3 (gate 2e-2; the
+-1 mask multiply is exact in bf16 - only the input cast rounds).

Rejected by measurement: fused 2MB DMAs (105us), SWDGE cast-in-DMA
(103us), alternating input DMAs across both HWDGE rings (106us), fat
15.9KB-descriptor row-pair layout (118us), 32 fine-grained tiles (104us),
bufs=5/6 (within drift), short-tail tiles (within drift).
"""

import sys

if "/opt/trn_rl_repo" not in sys.path:
    sys.path.insert(0, "/opt/trn_rl_repo")

import numpy as np

B, H, W, CH = 512, 64, 64, 31
N_CORES = 8
B_PER_CORE = B // N_CORES  # 64
B_PER_TILE = 2  # 2 batches x 64 rows = 128 partitions
N_TILES = B_PER_CORE // B_PER_TILE  # 32
FREE = W * CH  # 1984

TRACE = False

MASK_HALF_VARIANTS = {"v7c", "v7d", "v7e", "v7f", "v7g"}

_nc_cache: dict = {}


def _emit_body(tc, x, m, out, variant: str):
    """One full pass over the per-core shard."""
    import concourse.mybir as mybir

    nc = tc.nc
    f32 = mybir.dt.float32

    xv = x.rearrange("(t b) i w c -> t (b i) (w c)", b=B_PER_TILE)  # [32,128,1984]
    ov = out.rearrange("(t b) i w -> t (b i) w", b=B_PER_TILE)  # [32,128,64]

    with (
        tc.tile_pool(name="mconst", bufs=1) as mpool,
        tc.tile_pool(name="work", bufs=4) as pool,
    ):
        mt = mpool.tile([128, FREE], f32)
        nc.sync.dma_start(out=mt[:], in_=m)
        for t in range(N_TILES):
            xt = pool.tile([128, FREE], f32)
            nc.sync.dma_start(out=xt[:], in_=xv[t])
            if variant == "dma":  # DMA-in only: measures HBM read bandwidth
                nc.sync.dma_start(out=ov[t], in_=xt[:, :W])
                continue
            red = pool.tile([128, W], f32)
            if variant == "v1":
                prod = pool.tile([128, FREE], f32)
                nc.vector.tensor_mul(out=prod[:], in0=xt[:], in1=mt[:])
                nc.vector.reduce_sum(
                    out=red[:],
                    in_=prod[:].rearrange("p (w c) -> p w c", c=CH),
                    axis=mybir.AxisListType.X,
                )
            elif variant == "mult_only":  # multiply, skip reduce (wrong result)
                prod = pool.tile([128, FREE], f32)
                nc.vector.tensor_mul(out=prod[:], in0=xt[:], in1=mt[:])
                nc.vector.tensor_copy(out=red[:], in_=prod[:, : W])
            elif variant == "reduce_only":  # reduce, skip multiply (wrong result)
                nc.vector.reduce_sum(
                    out=red[:],
                    in_=xt[:].rearrange("p (w c) -> p w c", c=CH),
                    axis=mybir.AxisListType.X,
                )
            elif variant == "v2":  # multiply split DVE/GPSIMD, reduce on DVE
                prod = pool.tile([128, FREE], f32)
                eng = nc.vector if t % 2 == 0 else nc.gpsimd
                eng.tensor_mul(out=prod[:], in0=xt[:], in1=mt[:])
                nc.vector.reduce_sum(
                    out=red[:],
                    in_=prod[:].rearrange("p (w c) -> p w c", c=CH),
                    axis=mybir.AxisListType.X,
                )
            else:
                raise ValueError(variant)
            nc.sync.dma_start(out=ov[t], in_=red[:])


def _emit_body_v3(tc, x, m2, out, in_place: bool, out_ring=None, bufs=4):
    """Fused tiles: 4 batches per tile ([128, 3968]), one mult + one reduce."""
    import concourse.mybir as mybir

    nc = tc.nc
    f32 = mybir.dt.float32
    bpt = 4  # batches per fused tile
    n_tiles = B_PER_CORE // bpt  # 16
    if out_ring is None:
        out_ring = nc.sync

    # [16, 128, 2, 1984]: tile t covers batches 4t..4t+3; partition=(b%2, i)
    # via (g b) with g the outer pair index inside the tile
    xv = x.rearrange("(t g b) i w c -> t (b i) g (w c)", g=2, b=2)
    ov = out.rearrange("(t g b) i w -> t (b i) g w", g=2, b=2)

    with (
        tc.tile_pool(name="mconst", bufs=1) as mpool,
        tc.tile_pool(name="work", bufs=bufs) as pool,
        tc.tile_pool(name="red", bufs=4) as rpool,
    ):
        mt = mpool.tile([128, 2 * FREE], f32)
        nc.sync.dma_start(out=mt[:], in_=m2)
        for t in range(n_tiles):
            xt = pool.tile([128, 2 * FREE], f32)
            xtv = xt[:].rearrange("p (g f) -> p g f", g=2)
            # two 1MB DMAs per fused tile
            nc.sync.dma_start(out=xtv[:, 0], in_=xv[t, :, 0])
            nc.sync.dma_start(out=xtv[:, 1], in_=xv[t, :, 1])
            if in_place:
                prodap = xt[:]
            else:
                prod = pool.tile([128, 2 * FREE], f32)
                prodap = prod[:]
            nc.vector.tensor_mul(out=prodap, in0=xt[:], in1=mt[:])
            red = rpool.tile([128, 2 * W], f32)
            nc.vector.reduce_sum(
                out=red[:].rearrange("p (g w) -> p g w", g=2),
                in_=prodap.rearrange("p (g w c) -> p g w c", g=2, c=CH),
                axis=mybir.AxisListType.X,
            )
            out_ring.dma_start(
                out=ov[t], in_=red[:].rearrange("p (g w) -> p g w", g=2)
            )


def _emit_body_v7(
    tc,
    x,
    m,
    out,
    cast: str,
    bufs: int = 4,
    fused_dma: bool = False,
    mask_half: bool = False,
    mask_ring: str = "sync",
    alt_rings: bool = False,
    in_place: bool = False,
    split_cast: bool = False,
    split_compute: bool = False,
):
    """bf16 path: cast x fp32->bf16 off-DVE, then DVE bf16 mul (2x) + reduce (2x).

    cast="act": HWDGE fp32 DMA in (sync ring) + ACT copy-cast to bf16.
    cast="dma": SWDGE casting DMA in (gpsimd ring) straight to bf16.
    Reduce output is bf16 (2x mode requires all operands 2-byte); a tiny ACT
    cast converts [128, 128] back to fp32 before the scalar-ring out DMA.
    """
    import concourse.mybir as mybir

    nc = tc.nc
    f32 = mybir.dt.float32
    bf16 = mybir.dt.bfloat16
    n_tiles = B_PER_CORE // 4  # 16 fused tiles of 4 batches

    xv = x.rearrange("(t g b) i w c -> t (b i) g (w c)", g=2, b=2)
    ov = out.rearrange("(t g b) i w -> t (b i) g w", g=2, b=2)

    with (
        tc.tile_pool(name="mconst", bufs=1) as mpool,
        tc.tile_pool(name="xf", bufs=bufs) as xfpool,
        tc.tile_pool(name="xb", bufs=3) as xbpool,
        tc.tile_pool(name="prod", bufs=3) as ppool,
        tc.tile_pool(name="red", bufs=4) as rpool,
    ):
        if cast not in ("floor", "floor2"):
            mt = mpool.tile([128, FREE if mask_half else 2 * FREE], bf16)
            mring = nc.scalar if mask_ring == "scalar" else nc.sync
            mring.dma_start(out=mt[:], in_=m)
        for t in range(n_tiles):
            if cast == "floor":  # pure DMA floor: fused 2MB in, 64KB out
                xt = xfpool.tile([128, 2 * FREE], f32)
                xtv = xt[:].rearrange("p (g f) -> p g f", g=2)
                nc.sync.dma_start(out=xtv, in_=xv[t])
                nc.scalar.dma_start(out=ov[t], in_=xtv[:, :, :W])
                continue
            if cast == "floor2":  # v7's exact DMA pattern, no compute
                xt = xfpool.tile([128, 2 * FREE], f32)
                xtv = xt[:].rearrange("p (g f) -> p g f", g=2)
                nc.sync.dma_start(out=xtv[:, 0], in_=xv[t, :, 0])
                nc.sync.dma_start(out=xtv[:, 1], in_=xv[t, :, 1])
                nc.scalar.dma_start(out=ov[t], in_=xtv[:, :, :W])
                continue
            xb = xbpool.tile([128, 2 * FREE], bf16)
            if cast == "act":
                xt = xfpool.tile([128, 2 * FREE], f32)
                xtv = xt[:].rearrange("p (g f) -> p g f", g=2)
                if fused_dma:
                    nc.sync.dma_start(out=xtv, in_=xv[t])
                elif alt_rings:
                    nc.sync.dma_start(out=xtv[:, 0], in_=xv[t, :, 0])
                    nc.scalar.dma_start(out=xtv[:, 1], in_=xv[t, :, 1])
                else:
                    nc.sync.dma_start(out=xtv[:, 0], in_=xv[t, :, 0])
                    nc.sync.dma_start(out=xtv[:, 1], in_=xv[t, :, 1])
                if split_cast:  # cast each 1MB half as soon as it lands
                    xbv = xb[:].rearrange("p (g f) -> p g f", g=2)
                    nc.scalar.copy(out=xbv[:, 0], in_=xtv[:, 0])
                    nc.scalar.copy(out=xbv[:, 1], in_=xtv[:, 1])
                else:
                    nc.scalar.copy(out=xb[:], in_=xt[:])
            elif cast == "dma":
                xbv = xb[:].rearrange("p (g f) -> p g f", g=2)
                if fused_dma:
                    nc.gpsimd.dma_start(out=xbv, in_=xv[t])
                else:
                    nc.gpsimd.dma_start(out=xbv[:, 0], in_=xv[t, :, 0])
                    nc.gpsimd.dma_start(out=xbv[:, 1], in_=xv[t, :, 1])
            else:
                raise ValueError(cast)
            if in_place:
                prodap = xb[:]
            else:
                prod = ppool.tile([128, 2 * FREE], bf16)
                prodap = prod[:]
            if mask_half:
                pv = prodap.rearrange("p (g f) -> p g f", g=2)
                xbv2 = xb[:].rearrange("p (g f) -> p g f", g=2)
                nc.vector.tensor_mul(out=pv[:, 0], in0=xbv2[:, 0], in1=mt[:])
                nc.vector.tensor_mul(out=pv[:, 1], in0=xbv2[:, 1], in1=mt[:])
            elif split_compute:
                pv = prodap.rearrange("p (g f) -> p g f", g=2)
                xbv2 = xb[:].rearrange("p (g f) -> p g f", g=2)
                mtv = mt[:].rearrange("p (g f) -> p g f", g=2)
                nc.vector.tensor_mul(out=pv[:, 0], in0=xbv2[:, 0], in1=mtv[:, 0])
                nc.vector.tensor_mul(out=pv[:, 1], in0=xbv2[:, 1], in1=mtv[:, 1])
            else:
                nc.vector.tensor_mul(out=prodap, in0=xb[:], in1=mt[:])
            red = rpool.tile([128, 2 * W], bf16)
            with nc.allow_low_precision(reason="bf16 store of 31-term sum; DVE accumulates fp32 internally"):
                if split_compute:  # per-half reduce: shorter tail chain
                    pv2 = prodap.rearrange("p (g w c) -> p g w c", g=2, c=CH)
                    rv = red[:].rearrange("p (g w) -> p g w", g=2)
                    nc.vector.reduce_sum(
                        out=rv[:, 0], in_=pv2[:, 0], axis=mybir.AxisListType.X
                    )
                    nc.vector.reduce_sum(
                        out=rv[:, 1], in_=pv2[:, 1], axis=mybir.AxisListType.X
                    )
                else:
                    nc.vector.reduce_sum(
                        out=red[:].rearrange("p (g w) -> p g w", g=2),
                        in_=prodap.rearrange("p (g w c) -> p g w c", g=2, c=CH),
                        axis=mybir.AxisListType.X,
                    )
            redf = rpool.tile([128, 2 * W], f32)
            nc.scalar.copy(out=redf[:], in_=red[:])
            oring = nc.sync if (alt_rings and t % 2 == 0) else nc.scalar
            oring.dma_start(
                out=ov[t], in_=redf[:].rearrange("p (g w) -> p g w", g=2)
            )


def _emit_body_v7f(tc, x, m, out, bufs: int = 6):
    """v7 pipeline at 2-batch granularity: 32 tiles, one 1MB DMA each.

    Finer tiles shrink pipeline fill/drain; mask is [128, FREE] (no g dim).
    """
    import concourse.mybir as mybir

    nc = tc.nc
    f32 = mybir.dt.float32
    bf16 = mybir.dt.bfloat16

    xv = x.rearrange("(t b) i w c -> t (b i) (w c)", b=2)  # [32,128,1984]
    ov = out.rearrange("(t b) i w -> t (b i) w", b=2)  # [32,128,64]

    with (
        tc.tile_pool(name="mconst", bufs=1) as mpool,
        tc.tile_pool(name="xf", bufs=bufs) as xfpool,
        tc.tile_pool(name="xb", bufs=4) as xbpool,
        tc.tile_pool(name="prod", bufs=4) as ppool,
        tc.tile_pool(name="red", bufs=6) as rpool,
    ):
        mt = mpool.tile([128, FREE], bf16)
        nc.scalar.dma_start(out=mt[:], in_=m)
        for t in range(N_TILES):
            xt = xfpool.tile([128, FREE], f32)
            nc.sync.dma_start(out=xt[:], in_=xv[t])
            xb = xbpool.tile([128, FREE], bf16)
            nc.scalar.copy(out=xb[:], in_=xt[:])
            prod = ppool.tile([128, FREE], bf16)
            nc.vector.tensor_mul(out=prod[:], in0=xb[:], in1=mt[:])
            red = rpool.tile([128, W], bf16)
            with nc.allow_low_precision(reason="bf16 store; DVE accumulates fp32"):
                nc.vector.reduce_sum(
                    out=red[:],
                    in_=prod[:].rearrange("p (w c) -> p w c", c=CH),
                    axis=mybir.AxisListType.X,
                )
            redf = rpool.tile([128, W], f32)
            nc.scalar.copy(out=redf[:], in_=red[:])
            nc.scalar.dma_start(out=ov[t], in_=redf[:])


def _emit_body_v7t(tc, x, m, out, bufs: int = 4):
    """v7 (act-cast) with the last fused tile split into two 2-batch tiles,
    shortening the serial cast->mul->reduce->out tail after the final DMA."""
    import concourse.mybir as mybir

    nc = tc.nc
    f32 = mybir.dt.float32
    bf16 = mybir.dt.bfloat16

    pairs = x.rearrange("(pair b) i w c -> pair (b i) (w c)", b=2)  # [32,128,1984]
    opairs = out.rearrange("(pair b) i w -> pair (b i) w", b=2)  # [32,128,64]
    ov4 = out.rearrange("(t g b) i w -> t (b i) g w", g=2, b=2)  # [16,128,2,64]

    with (
        tc.tile_pool(name="mconst", bufs=1) as mpool,
        tc.tile_pool(name="xf", bufs=bufs) as xfpool,
        tc.tile_pool(name="xb", bufs=3) as xbpool,
        tc.tile_pool(name="prod", bufs=3) as ppool,
        tc.tile_pool(name="red", bufs=4) as rpool,
    ):
        mt = mpool.tile([128, 2 * FREE], bf16)
        nc.scalar.dma_start(out=mt[:], in_=m)
        for t in range(15):  # 4-batch fused tiles, pairs (2t, 2t+1)
            xt = xfpool.tile([128, 2 * FREE], f32)
            xtv = xt[:].rearrange("p (g f) -> p g f", g=2)
            nc.sync.dma_start(out=xtv[:, 0], in_=pairs[2 * t])
            nc.sync.dma_start(out=xtv[:, 1], in_=pairs[2 * t + 1])
            xb = xbpool.tile([128, 2 * FREE], bf16)
            nc.scalar.copy(out=xb[:], in_=xt[:])
            prod = ppool.tile([128, 2 * FREE], bf16)
            nc.vector.tensor_mul(out=prod[:], in0=xb[:], in1=mt[:])
            red = rpool.tile([128, 2 * W], bf16)
            with nc.allow_low_precision(reason="bf16 store; DVE accumulates fp32"):
                nc.vector.reduce_sum(
                    out=red[:].rearrange("p (g w) -> p g w", g=2),
                    in_=prod[:].rearrange("p (g w c) -> p g w c", g=2, c=CH),
                    axis=mybir.AxisListType.X,
                )
            redf = rpool.tile([128, 2 * W], f32)
            nc.scalar.copy(out=redf[:], in_=red[:])
            nc.scalar.dma_start(
                out=ov4[t], in_=redf[:].rearrange("p (g w) -> p g w", g=2)
            )
        for pr in (30, 31):  # 2-batch tail tiles
            xt = xfpool.tile([128, FREE], f32)
            nc.sync.dma_start(out=xt[:], in_=pairs[pr])
            xb = xbpool.tile([128, FREE], bf16)
            nc.scalar.copy(out=xb[:], in_=xt[:])
            prod = ppool.tile([128, FREE], bf16)
            nc.vector.tensor_mul(out=prod[:], in0=xb[:], in1=mt[:, :FREE])
            red = rpool.tile([128, W], bf16)
            with nc.allow_low_precision(reason="bf16 store; DVE accumulates fp32"):
                nc.vector.reduce_sum(
                    out=red[:],
                    in_=prod[:].rearrange("p (w c) -> p w c", c=CH),
                    axis=mybir.AxisListType.X,
                )
            redf = rpool.tile([128, W], f32)
            nc.scalar.copy(out=redf[:], in_=red[:])
            nc.scalar.dma_start(out=opairs[pr], in_=redf[:])


def _emit_body_v7h(tc, x, m, out, bufs: int = 4, floor: bool = False):
    """Row-pair layout: partition=(b4, rowpair32), free=(row_lo2, w, c).

    Each partition's DRAM run is 2 rows x 1984 x 4B = 15.9KB contiguous, so a
    tile is ONE clean 2D [128, 3968] DMA with 128 fat descriptors (vs 256 thin
    ones in the v7 layout). Mask content depends on (rowpair, row_lo) but not
    b, still [128, 2*FREE] bf16.
    """
    import concourse.mybir as mybir

    nc = tc.nc
    f32 = mybir.dt.float32
    bf16 = mybir.dt.bfloat16
    n_tiles = B_PER_CORE // 4  # 16

    xv = x.rearrange("(t b) (q l) w c -> t (b q) (l w c)", b=4, l=2)  # [16,128,3968]
    ov = out.rearrange("(t b) (q l) w -> t (b q) (l w)", b=4, l=2)  # [16,128,128]

    with (
        tc.tile_pool(name="mconst", bufs=1) as mpool,
        tc.tile_pool(name="xf", bufs=bufs) as xfpool,
        tc.tile_pool(name="xb", bufs=3) as xbpool,
        tc.tile_pool(name="prod", bufs=3) as ppool,
        tc.tile_pool(name="red", bufs=4) as rpool,
    ):
        if not floor:
            mt = mpool.tile([128, 2 * FREE], bf16)
            nc.scalar.dma_start(out=mt[:], in_=m)
        for t in range(n_tiles):
            xt = xfpool.tile([128, 2 * FREE], f32)
            nc.sync.dma_start(out=xt[:], in_=xv[t])
            if floor:
                nc.scalar.dma_start(out=ov[t], in_=xt[:, : 2 * W])
                continue
            xb = xbpool.tile([128, 2 * FREE], bf16)
            nc.scalar.copy(out=xb[:], in_=xt[:])
            prod = ppool.tile([128, 2 * FREE], bf16)
            nc.vector.tensor_mul(out=prod[:], in0=xb[:], in1=mt[:])
            red = rpool.tile([128, 2 * W], bf16)
            with nc.allow_low_precision(reason="bf16 store; DVE accumulates fp32"):
                nc.vector.reduce_sum(
                    out=red[:],
                    in_=prod[:].rearrange("p (lw c) -> p lw c", c=CH),
                    axis=mybir.AxisListType.X,
                )
            redf = rpool.tile([128, 2 * W], f32)
            nc.scalar.copy(out=redf[:], in_=red[:])
            nc.scalar.dma_start(out=ov[t], in_=redf[:])


def build_nc(variant: str = "v1", reps: int = 1):
    key = (variant, reps)
    if key in _nc_cache:
        return _nc_cache[key]

    import concourse.bacc as bacc
    import concourse.mybir as mybir
    import concourse.tile as tile

    f32 = mybir.dt.float32
    nc = bacc.Bacc("TRN2", debug=False, num_devices=N_CORES)
    x = nc.dram_tensor("x", [B_PER_CORE, H, W, CH], f32, kind="ExternalInput").ap()
    mask_half = variant in MASK_HALF_VARIANTS
    m_free = FREE if mask_half else (
        2 * FREE
        if variant.startswith(("v3", "v4", "v5", "v6", "v7", "v8"))
        or variant == "dma3"
        else FREE
    )
    m_dt = (
        mybir.dt.bfloat16
        if variant.startswith(("v7", "v8")) or variant == "dma3"
        else f32
    )
    m = nc.dram_tensor("m", [128, m_free], m_dt, kind="ExternalInput").ap()
    out = nc.dram_tensor("out", [B_PER_CORE, H, W], f32, kind="ExternalOutput").ap()

    with tile.TileContext(nc) as tc:
        for _ in range(reps):
            if variant == "v7":
                _emit_body_v7(tc, x, m, out, cast="act")
            elif variant == "v7b":
                _emit_body_v7(tc, x, m, out, cast="act", fused_dma=True)
            elif variant == "v7c":
                _emit_body_v7(
                    tc, x, m, out, cast="act", mask_half=True, mask_ring="scalar"
                )
            elif variant == "v7d":
                _emit_body_v7(
                    tc, x, m, out, cast="act", mask_half=True,
                    mask_ring="scalar", alt_rings=True,
                )
            elif variant == "v7e":
                _emit_body_v7(
                    tc, x, m, out, cast="act", mask_half=True,
                    mask_ring="scalar", bufs=6,
                )
            elif variant == "v7f":
                _emit_body_v7f(tc, x, m, out)
            elif variant == "v7g":
                _emit_body_v7f(tc, x, m, out, bufs=8)
            elif variant == "v7h":
                _emit_body_v7h(tc, x, m, out)
            elif variant == "v7t":
                _emit_body_v7t(tc, x, m, out)
            elif variant == "v7j":
                _emit_body_v7(tc, x, m, out, cast="act", bufs=5)
            elif variant == "v7ip":
                _emit_body_v7(tc, x, m, out, cast="act", in_place=True)
            elif variant == "v7ip2":
                _emit_body_v7(tc, x, m, out, cast="act", in_place=True, bufs=6)
            elif variant == "v7ip3":
                _emit_body_v7(
                    tc, x, m, out, cast="act", in_place=True, bufs=6,
                    split_cast=True,
                )
            elif variant == "v7ip4":
                _emit_body_v7(
                    tc, x, m, out, cast="act", in_place=True, bufs=6,
                    split_cast=True, split_compute=True,
                )
            elif variant == "dma4":
                _emit_body_v7(tc, x, m, out, cast="floor2")
            elif variant == "dma3":
                _emit_body_v7h(tc, x, m, out, floor=True)
            elif variant == "v8":
                _emit_body_v7(tc, x, m, out, cast="dma")
            elif variant == "v8b":
                _emit_body_v7(tc, x, m, out, cast="dma", fused_dma=True)
            elif variant == "dma2":
                _emit_body_v7(tc, x, m, out, cast="floor", fused_dma=True)
            elif variant == "v4":
                _emit_body_v3(tc, x, m, out, in_place=False, out_ring=nc.scalar)
            elif variant == "v5":
                _emit_body_v3(
                    tc, x, m, out, in_place=False, out_ring=nc.scalar, bufs=6
                )
            elif variant == "v6":
                _emit_body_v3(tc, x, m, out, in_place=False, bufs=6)
            elif variant.startswith(("v3", "v4", "v5")):
                _emit_body_v3(tc, x, m, out, in_place=variant == "v3ip")
            else:
                _emit_body(tc, x, m, out, variant)

    nc.compile()
    _nc_cache[key] = nc
    return nc


def host_sign_tensor(w: np.ndarray) -> np.ndarray:
    """M_rep[p, w*31+ch] = mask32[((p%64)-ch)%32, w%32], shape [128, 1984] f32."""
    mask32 = np.sign(w.astype(np.float32)).reshape(32, 32)
    i_idx = np.arange(H)
    ch_idx = np.arange(CH)
    rel = (i_idx[:, None] - ch_idx[None, :]) % 32  # [64, 31]
    w_mod = np.arange(W) % 32
    M = mask32[rel[:, None, :], w_mod[None, :, None]]  # [64, 64, 31]
    M = np.ascontiguousarray(M.reshape(H, FREE), dtype=np.float32)
    return np.tile(M, (B_PER_TILE, 1))  # [128, 1984]


VARIANT = "v7ip3"


def prepare_mask(variant: str, w: np.ndarray) -> np.ndarray:
    """Host-side mask tensor in the layout/dtype the variant expects."""
    import concourse.mybir as mybir

    if variant in ("v7h", "dma3"):
        # partition=(b4, q32), free=(l2, w, c): M[q, l*FREE + w*31 + ch]
        #   = mask32[(2q + l - ch) % 32, w % 32], replicated over b.
        mask32 = np.sign(np.asarray(w).astype(np.float32)).reshape(32, 32)
        q = np.arange(32)
        l = np.arange(2)
        ch = np.arange(CH)
        rel = (2 * q[:, None, None] + l[None, :, None] - ch[None, None, :]) % 32
        w_mod = np.arange(W) % 32
        M = mask32[rel[:, :, None, :], w_mod[None, None, :, None]]  # [32,2,64,31]
        M = np.tile(M.reshape(32, 2 * FREE), (4, 1))  # [128, 3968]
        return M.astype(mybir.dt.np(mybir.dt.bfloat16))

    m_rep = host_sign_tensor(np.asarray(w))
    if variant not in MASK_HALF_VARIANTS and variant.startswith(
        ("v3", "v4", "v5", "v6", "v7", "v8")
    ):
        m_rep = np.tile(m_rep, (1, 2))  # [128, 3968]
    if variant.startswith(("v7", "v8")):
        m_rep = m_rep.astype(mybir.dt.np(mybir.dt.bfloat16))
    return m_rep


def kernel(x: np.ndarray, w: np.ndarray) -> np.ndarray:
    from concourse.bass_utils import run_bass_kernel_spmd

    x = np.ascontiguousarray(np.asarray(x), dtype=np.float32)
    m_rep = prepare_mask(VARIANT, np.asarray(w))

    nc = build_nc(VARIANT, 1)
    in_maps = [
        {"x": x[c * B_PER_CORE : (c + 1) * B_PER_CORE], "m": m_rep}
        for c in range(N_CORES)
    ]
    res = run_bass_kernel_spmd(nc, in_maps, core_ids=list(range(N_CORES)), trace=TRACE)
    if TRACE and res.exec_time_ns is not None:
        kernel.last_exec_time_ns = res.exec_time_ns
    return np.concatenate([r["out"] for r in res.results], axis=0)


kernel.last_exec_time_ns = None

